# revision 24
# baseline (speedup 1.0000x reference)
"""Trainium2 Bass kernel for nn_Block1_54279796687228 (retrieval_knn).

Math: the reference builds the full per-sample Jacobian J of the conv
encoder and contracts it with x.  For a conv+ReLU (piecewise-linear)
encoder, einsum(x, J) is exactly the JVP of the encoder at x in
direction x:

    z_q = m2 * conv2_nobias(m1 * conv1_nobias(x)),
    m1 = [conv1(x)+b1 > 0],  m2 = [conv2(relu(conv1(x)+b1))+b2 > 0]

With the zero biases produced by setup_inputs() this collapses to the
plain forward pass relu(conv2(relu(conv1(x)))).  Both variants are
implemented; the host picks based on the actual bias values.

Lowering:
  conv1 -> one K=48 matmul over a host-built im2col (layout only).
  conv2 -> fold (ci,kw) into K=128: ReLU+shift fused into 4
           tensor_scalar_max ops straight out of PSUM, then 4
           accumulating matmuls (one per kh).
  Hopfield -> scores are computed directly TRANSPOSED, (mem, pos), as
           4 matmuls with lkT chunks stationary — no softmax-axis
           transpose is ever needed.  One exp over the PSUM tile gives
           unnormalized E; the lookup chunks rebuilt on device carry
           an appended ones-column, so the 4 accumulating G matmuls
           produce [G; Z] in one go (Z = softmax denominator).  Z is
           transposed to a per-partition column by a trivial K=1
           matmul, and the 1/Z scale rides the final PSUM->SBUF copy.
           out2 = (G.T @ (Wv@Wo)) / Z, emitted (pos, ch'); the host
           transposes each (64,64) sample for free.  Wv@Wo is folded
           on device, early, off the critical path.

All matmuls run in float32r (single pass); ~2.7e-4 relative error
end-to-end vs the fp32 reference.

Sharding: pure data parallel over batch. Sample b runs on cores b and
b+4 (duplicates); host gathers from cores 0-3. Input DMAs are spread
across both HWDGE queues (sync, scalar) and the SWDGE queue (gpsimd),
ordered by when they gate compute.
"""

import os
import numpy as np

# -- NTFF profile hook shim -------------------------------------------------
# bass_utils' trace path needs antenv.axon_hooks, which this image's antenv
# lacks. Register the ctypes-based hook from trn_agent_boot if available so
# trace=True / BASS_TRACE=1 works; degrade silently otherwise.
def _ensure_ntff_hook():
    try:
        import antenv.axon_hooks  # noqa: F401
        return
    except ImportError:
        pass
    try:
        import sys, types
        import antenv
        from trn_agent_boot.trn_boot import _ntff_profile_via_ctypes

        mod = types.ModuleType("antenv.axon_hooks")
        _h = [None]
        mod.set_axon_ntff_profile_hook = lambda h: _h.__setitem__(0, h)
        mod.get_axon_ntff_profile_hook = lambda: _h[0]
        sys.modules["antenv.axon_hooks"] = mod
        antenv.axon_hooks = mod
        so = "/opt/axon/libaxon_pjrt.so"
        if os.path.exists(so):
            mod.set_axon_ntff_profile_hook(_ntff_profile_via_ctypes(so))
    except Exception:
        pass


_ensure_ntff_hook()

import concourse.bacc as bacc
import concourse.bass as bass
import concourse.tile as tile
import concourse.mybir as mybir
from concourse.bass_utils import run_bass_kernel_spmd

F32 = mybir.dt.float32
F32R = mybir.dt.float32r

B, CIN, C1, C2, Q = 4, 3, 32, 64, 512  # batch, in-ch, conv1-ch, conv2-ch, memories
N_CORES = 8

_COMPILED = {}  # variant -> nc
last_exec_time_ns = None
last_trace_path = None


def _build(with_bias: bool):
    nc = bacc.Bacc("TRN2", target_bir_lowering=False, debug=False,
                   enable_asserts=False)

    x_im = nc.dram_tensor("x_im", [48, 256], F32R, kind="ExternalInput")
    w1r = nc.dram_tensor("w1r", [48, 32], F32R, kind="ExternalInput")
    w2k = nc.dram_tensor("w2k", [128, 4, 64], F32R, kind="ExternalInput")
    lkT = nc.dram_tensor("lkT", [64, 512], F32R, kind="ExternalInput")
    wvT = nc.dram_tensor("wvT", [64, 64], F32R, kind="ExternalInput")
    ident_d = nc.dram_tensor("ident", [64, 64], F32R, kind="ExternalInput")
    wo = nc.dram_tensor("wo", [64, 64], F32R, kind="ExternalInput")
    if with_bias:
        b1 = nc.dram_tensor("b1", [32, 1], F32, kind="ExternalInput")
        b2 = nc.dram_tensor("b2", [64, 1], F32, kind="ExternalInput")
    out_d = nc.dram_tensor("out", [64, 64], F32, kind="ExternalOutput")

    with tile.TileContext(nc) as tc:
        with (
            tc.tile_pool(name="consts", bufs=1) as consts,
            tc.tile_pool(name="work", bufs=1) as work,
            tc.tile_pool(name="psA", bufs=1, space="PSUM") as psA,
            tc.tile_pool(name="psT", bufs=4 if not with_bias else 2, space="PSUM") as psT,
        ):
            # ---- loads, spread across the two HWDGE queues (sync, scalar)
            # and the SWDGE queue (gpsimd); ordered by when they gate compute.
            sb_xim = consts.tile([48, 256], F32R, tag="xim")
            nc.sync.dma_start(sb_xim[:32, :], x_im.ap()[:32, :])
            nc.scalar.dma_start(sb_xim[32:, :], x_im.ap()[32:, :])
            ident = consts.tile([64, 64], F32R, tag="ident")
            nc.gpsimd.dma_start(ident[:], ident_d.ap())
            sb_w1 = consts.tile([48, 32], F32R, tag="w1")
            nc.gpsimd.dma_start(sb_w1[:], w1r.ap())
            sb_w2 = consts.tile([128, 4, 64], F32R, tag="w2")
            nc.sync.dma_start(sb_w2[:, :2, :], w2k.ap()[:, :2, :])
            nc.scalar.dma_start(sb_w2[:, 2:, :], w2k.ap()[:, 2:, :])
            sb_lkT = consts.tile([64, 512], F32R, tag="lkT")
            nc.gpsimd.dma_start(sb_lkT[:, :256], lkT.ap()[:, :256])
            nc.sync.dma_start(sb_lkT[:, 256:], lkT.ap()[:, 256:])
            sb_wvT = consts.tile([64, 64], F32R, tag="wvT")
            nc.gpsimd.dma_start(sb_wvT[:], wvT.ap())
            sb_wo = consts.tile([64, 64], F32R, tag="wo")
            nc.scalar.dma_start(sb_wo[:], wo.ap())
            if with_bias:
                sb_b1 = consts.tile([32, 1], F32, tag="b1")
                nc.gpsimd.dma_start(sb_b1[:], b1.ap())
                sb_b2 = consts.tile([64, 1], F32, tag="b2")
                nc.gpsimd.dma_start(sb_b2[:], b2.ap())

            # f32r tiles cannot be memset directly; zero/one them via ops
            # from an f32 zero tile (early, no dependencies).
            sb_zero = consts.tile([128, 18, 8], F32, tag="zero")
            nc.vector.memset(sb_zero[:], 0.0)
            sb_one = consts.tile([65, 2], F32R, tag="one")
            nc.vector.tensor_scalar_add(sb_one[64:65, :], sb_zero[64:65, 0, :2], 1.0)

            sb_lk = work.tile([128, 4, 65], F32R, tag="lk")
            nc.vector.tensor_scalar_add(sb_lk[:, :, 64:65],
                                        sb_zero[:, :4, :1], 1.0)

            # ---- conv1: (48,32).T @ (48,256) -> (32, 16, 16) ----
            p_z1 = psA.tile([32, 16, 16], F32, tag="a")
            nc.tensor.matmul(p_z1[:], sb_w1[:], sb_xim[:],
                             start=True, stop=True)

            # ---- conv2 input: imkw[(kw,ci), row, c] = a1pad[ci, row, 2c+kw]
            # where a1pad = zero-pad(relu(z1)).  The ReLU, the shift and the
            # fp32r cast fuse into one tensor_scalar_max per kw, straight
            # from PSUM; pad rows/cols come from the early zero cast-copy.
            def build_imkw(src_psum, imkw):
                nc.vector.tensor_scalar_max(
                    imkw[0:32, 1:17, 1:8], src_psum[:, :, 1:15:2], 0.0)
                nc.vector.tensor_scalar_max(
                    imkw[32:64, 1:17, 0:8], src_psum[:, :, 0:16:2], 0.0)
                nc.vector.tensor_scalar_max(
                    imkw[64:96, 1:17, 0:8], src_psum[:, :, 1:16:2], 0.0)
                nc.vector.tensor_scalar_max(
                    imkw[96:128, 1:17, 0:7], src_psum[:, :, 2:16:2], 0.0)

            def conv2(imkw, ps_tag):
                p = psA.tile([64, 64], F32, tag=ps_tag)
                for kh in range(4):
                    nc.tensor.matmul(
                        p[:],
                        sb_w2[:, kh, :],
                        imkw[:, kh:min(kh + 16, 18):2, :],
                        start=(kh == 0), stop=(kh == 3),
                    )
                return p

            imkw = work.tile([128, 18, 8], F32R, tag="imkw")
            nc.vector.tensor_copy(imkw[:], sb_zero[:])
            if not with_bias:
                build_imkw(p_z1, imkw)
                p_z2 = conv2(imkw, "b")
            else:
                # a1 = relu(z1 + b1); t1m = z1 * sign(a1)
                sb_a1 = work.tile([32, 16, 16], F32, tag="a1")
                nc.scalar.activation(
                    sb_a1[:], p_z1[:], mybir.ActivationFunctionType.Relu,
                    bias=sb_b1[:], scale=1.0,
                )
                sb_m1 = work.tile([32, 16, 16], F32, tag="m1")
                nc.scalar.activation(
                    sb_m1[:], sb_a1[:], mybir.ActivationFunctionType.Sign)
                sb_t1 = work.tile([32, 16, 16], F32, tag="t1")
                nc.vector.tensor_mul(sb_t1[:], p_z1[:], sb_m1[:])

                def shifts(dst, src):
                    nc.vector.tensor_copy(dst[0:32, 1:17, 1:8], src[:, :, 1:15:2])
                    nc.vector.tensor_copy(dst[32:64, 1:17, 0:8], src[:, :, 0:16:2])
                    nc.vector.tensor_copy(dst[64:96, 1:17, 0:8], src[:, :, 1:16:2])
                    nc.vector.tensor_copy(dst[96:128, 1:17, 0:7], src[:, :, 2:16:2])

                shifts(imkw, sb_a1)
                p_z2 = conv2(imkw, "b")
                imkw2 = work.tile([128, 18, 8], F32R, tag="imkw2")
                nc.vector.tensor_copy(imkw2[:], sb_zero[:])
                shifts(imkw2, sb_t1)
                p_t2 = conv2(imkw2, "e")

            sb_zq = work.tile([64, 64], F32R, tag="zq")
            if not with_bias:
                nc.vector.tensor_scalar_max(sb_zq[:], p_z2[:], 0.0)
            else:
                sb_z2r = work.tile([64, 64], F32, tag="z2r")
                nc.scalar.activation(
                    sb_z2r[:], p_z2[:], mybir.ActivationFunctionType.Relu,
                    bias=sb_b2[:], scale=1.0,
                )
                sb_m2 = work.tile([64, 64], F32, tag="m2")
                nc.scalar.activation(
                    sb_m2[:], sb_z2r[:], mybir.ActivationFunctionType.Sign)
                nc.vector.tensor_mul(sb_zq[:], p_t2[:], sb_m2[:])

            # ---- natural-layout lookup chunks from lkT via PE transpose
            # (interleaved with scoresT by DMA-arrival order), plus
            # scoresT: 4 matmuls, (mem128, pos) chunks side by side in one
            # PSUM tile; lkT chunk is the stationary operand.  The lk
            # chunks carry an appended ones-column (written above) so the
            # G matmuls also emit Z.
            p_sT = psA.tile([128, 4, 64], F32, tag="c")
            for c in range(4):
                nc.tensor.matmul(
                    p_sT[:, c, :],
                    sb_lkT[:, 128 * c:128 * (c + 1)], sb_zq[:],
                    start=True, stop=True,
                )
            p_lks = []
            for c in range(4):
                p_lk = psT.tile([128, 64], F32, tag="ptr")
                nc.tensor.matmul(
                    p_lk[:], sb_lkT[:, 128 * c:128 * (c + 1)], ident[:],
                    start=True, stop=True,
                )
                p_lks.append(p_lk)
                nc.scalar.copy(sb_lk[:, c, :64], p_lk[:])

            # ---- Wvo = Wv @ Wo (needed only by the final matmul); the PE
            # is otherwise idle while the exp runs.
            p_wvo = psA.tile([64, 64], F32, tag="d")
            nc.tensor.matmul(p_wvo[:], sb_wvT[:], sb_wo[:],
                             start=True, stop=True)
            sb_wvo = work.tile([64, 64], F32R, tag="wvo")
            nc.scalar.copy(sb_wvo[:], p_wvo[:])

            # unnormalized softmax: E = exp(s/8) over the whole tile.
            # |s/8| << 1 here, so max-subtraction is unnecessary in fp32.
            sb_E = work.tile([128, 4, 64], F32R, tag="E")
            nc.scalar.activation(
                sb_E[:], p_sT[:], mybir.ActivationFunctionType.Exp,
                scale=0.125,
            )

            # ---- [G; Z][d, pos] = sum_m [lk | 1][m, d] * E[m, pos] ----
            p_g = psA.tile([65, 64], F32, tag="d")
            for c in range(4):
                nc.tensor.matmul(
                    p_g[:], sb_lk[:, c, :], sb_E[:, c, :],
                    start=(c == 0), stop=(c == 3),
                )
            sb_g = work.tile([65, 64], F32R, tag="g")
            nc.vector.tensor_copy(sb_g[:], p_g[:])

            # Z row -> per-partition column via a K=1 matmul, then 1/Z
            p_zT = psA.tile([64, 2], F32, tag="b")
            nc.tensor.matmul(p_zT[:], sb_g[64:65, :].bitcast(F32), sb_one[64:65, :].bitcast(F32),
                             start=True, stop=True)
            sb_rz = work.tile([64, 1], F32, tag="rz")
            nc.vector.reciprocal(sb_rz[:], p_zT[:, :1])

            # ---- out2[pos, ch'] = (G.T @ Wvo)[pos, ch'] / Z[pos] ----
            p_o = psA.tile([64, 64], F32, tag="a")
            nc.tensor.matmul(p_o[:], sb_g[:64, :], sb_wvo[:],
                             start=True, stop=True)
            sb_out = work.tile([64, 64], F32, tag="out")
            nc.vector.tensor_scalar_mul(sb_out[:], p_o[:], sb_rz[:])
            nc.sync.dma_start(out_d.ap()[:32, :], sb_out[:32, :])
            nc.scalar.dma_start(out_d.ap()[32:, :], sb_out[32:, :])

    nc.compile()
    return nc


def _get_nc(with_bias: bool):
    if with_bias not in _COMPILED:
        _COMPILED[with_bias] = _build(with_bias)
    return _COMPILED[with_bias]


def kernel(x, conv1_w, conv1_b, conv2_w, conv2_b, lookup, Wv, Wo):
    global last_exec_time_ns, last_trace_path
    x = np.asarray(x, np.float32)
    w1 = np.asarray(conv1_w, np.float32)
    b1 = np.asarray(conv1_b, np.float32)
    w2 = np.asarray(conv2_w, np.float32)
    b2 = np.asarray(conv2_b, np.float32)
    lk = np.ascontiguousarray(np.asarray(lookup, np.float32))
    wv = np.ascontiguousarray(np.asarray(Wv, np.float32))
    wo = np.ascontiguousarray(np.asarray(Wo, np.float32))

    with_bias = bool(np.any(b1 != 0.0) or np.any(b2 != 0.0))

    # host-side layout prep (no arithmetic): im2col of padded x, weight
    # transposes to the matmul-native layouts.
    xp = np.zeros((B, CIN, 34, 34), np.float32)
    xp[:, :, 1:33, 1:33] = x
    xim = np.empty((B, CIN, 4, 4, 16, 16), np.float32)
    for kh in range(4):
        for kw in range(4):
            xim[:, :, kh, kw] = xp[:, :, kh:kh + 32:2, kw:kw + 32:2]
    xim = np.ascontiguousarray(xim.reshape(B, 48, 256))

    w1r = np.ascontiguousarray(w1.transpose(1, 2, 3, 0).reshape(48, 32))
    # w2k[(kw*32+ci), kh, co] = w2[co, ci, kh, kw]
    w2k = np.ascontiguousarray(w2.transpose(3, 1, 2, 0).reshape(128, 4, 64))
    lkT = np.ascontiguousarray(lk.T)
    wvT = np.ascontiguousarray(wv.T)

    shared = {"w1r": w1r, "w2k": w2k, "lkT": lkT, "wvT": wvT, "wo": wo,
              "ident": np.eye(64, dtype=np.float32)}
    if with_bias:
        shared["b1"] = np.ascontiguousarray(b1.reshape(32, 1))
        shared["b2"] = np.ascontiguousarray(b2.reshape(64, 1))

    in_maps = [dict(shared, x_im=xim[c % B]) for c in range(N_CORES)]

    nc = _get_nc(with_bias)
    trace = bool(os.environ.get("KERNEL_TRACE"))
    res = run_bass_kernel_spmd(
        nc, in_maps, core_ids=list(range(N_CORES)),
        trace=trace, trace_cores=[0] if trace else None,
    )
    last_exec_time_ns = res.exec_time_ns
    if res.instructions_and_trace:
        last_trace_path = res.instructions_and_trace[1]

    # device emits (pos, ch') per sample; host transposes (layout only)
    out = np.stack([res.results[b]["out"].T for b in range(B)])
    return np.ascontiguousarray(out.reshape(B, C2, 8, 8))


# revision 25
# speedup vs baseline: 1.0161x; 1.0161x over previous
"""Trainium2 Bass kernel for nn_Block1_54279796687228 (retrieval_knn).

Math: the reference builds the full per-sample Jacobian J of the conv
encoder and contracts it with x.  For a conv+ReLU (piecewise-linear)
encoder, einsum(x, J) is exactly the JVP of the encoder at x in
direction x:

    z_q = m2 * conv2_nobias(m1 * conv1_nobias(x)),
    m1 = [conv1(x)+b1 > 0],  m2 = [conv2(relu(conv1(x)+b1))+b2 > 0]

With the zero biases produced by setup_inputs() this collapses to the
plain forward pass relu(conv2(relu(conv1(x)))).  Both variants are
implemented; the host picks based on the actual bias values.

Lowering:
  conv1 -> one K=48 matmul over a host-built im2col (layout only).
  conv2 -> fold (ci,kw) into K=128: ReLU+shift fused into 4
           tensor_scalar_max ops straight out of PSUM, then 4
           accumulating matmuls (one per kh).
  Hopfield -> scores are computed directly TRANSPOSED, (mem, pos), as
           4 matmuls with lkT chunks stationary — no softmax-axis
           transpose is ever needed.  One exp over the PSUM tile gives
           unnormalized E; the lookup chunks rebuilt on device carry
           an appended ones-column, so the 4 accumulating G matmuls
           produce [G; Z] in one go (Z = softmax denominator).  Z is
           transposed to a per-partition column by a trivial K=1
           matmul, and the 1/Z scale rides the final PSUM->SBUF copy.
           out2 = (G.T @ (Wv@Wo)) / Z, emitted (pos, ch'); the host
           transposes each (64,64) sample for free.  Wv@Wo is folded
           on device, early, off the critical path.

All matmuls run in float32r (single pass); ~2.7e-4 relative error
end-to-end vs the fp32 reference.

Sharding: pure data parallel over batch. Sample b runs on cores b and
b+4 (duplicates); host gathers from cores 0-3. Input DMAs are spread
across both HWDGE queues (sync, scalar) and the SWDGE queue (gpsimd),
ordered by when they gate compute.
"""

import os
import numpy as np

# -- NTFF profile hook shim -------------------------------------------------
# bass_utils' trace path needs antenv.axon_hooks, which this image's antenv
# lacks. Register the ctypes-based hook from trn_agent_boot if available so
# trace=True / BASS_TRACE=1 works; degrade silently otherwise.
def _ensure_ntff_hook():
    try:
        import antenv.axon_hooks  # noqa: F401
        return
    except ImportError:
        pass
    try:
        import sys, types
        import antenv
        from trn_agent_boot.trn_boot import _ntff_profile_via_ctypes

        mod = types.ModuleType("antenv.axon_hooks")
        _h = [None]
        mod.set_axon_ntff_profile_hook = lambda h: _h.__setitem__(0, h)
        mod.get_axon_ntff_profile_hook = lambda: _h[0]
        sys.modules["antenv.axon_hooks"] = mod
        antenv.axon_hooks = mod
        so = "/opt/axon/libaxon_pjrt.so"
        if os.path.exists(so):
            mod.set_axon_ntff_profile_hook(_ntff_profile_via_ctypes(so))
    except Exception:
        pass


_ensure_ntff_hook()

import concourse.bacc as bacc
import concourse.bass as bass
import concourse.tile as tile
import concourse.mybir as mybir
from concourse.bass_utils import run_bass_kernel_spmd

F32 = mybir.dt.float32
F32R = mybir.dt.float32r

B, CIN, C1, C2, Q = 4, 3, 32, 64, 512  # batch, in-ch, conv1-ch, conv2-ch, memories
N_CORES = 8

_COMPILED = {}  # variant -> nc
last_exec_time_ns = None
last_trace_path = None


def _build(with_bias: bool):
    nc = bacc.Bacc("TRN2", target_bir_lowering=False, debug=False,
                   enable_asserts=False)

    x_im = nc.dram_tensor("x_im", [48, 256], F32R, kind="ExternalInput")
    w1r = nc.dram_tensor("w1r", [48, 32], F32R, kind="ExternalInput")
    w2k = nc.dram_tensor("w2k", [128, 4, 64], F32R, kind="ExternalInput")
    lkT = nc.dram_tensor("lkT", [64, 512], F32R, kind="ExternalInput")
    wvT = nc.dram_tensor("wvT", [64, 64], F32R, kind="ExternalInput")
    ident_d = nc.dram_tensor("ident", [64, 64], F32R, kind="ExternalInput")
    wo = nc.dram_tensor("wo", [64, 64], F32R, kind="ExternalInput")
    if with_bias:
        b1 = nc.dram_tensor("b1", [32, 1], F32, kind="ExternalInput")
        b2 = nc.dram_tensor("b2", [64, 1], F32, kind="ExternalInput")
    out_d = nc.dram_tensor("out", [64, 64], F32, kind="ExternalOutput")

    with tile.TileContext(nc) as tc:
        with (
            tc.tile_pool(name="consts", bufs=1) as consts,
            tc.tile_pool(name="work", bufs=1) as work,
            tc.tile_pool(name="psA", bufs=1, space="PSUM") as psA,
            tc.tile_pool(name="psT", bufs=4 if not with_bias else 2, space="PSUM") as psT,
        ):
            # ---- loads, spread across the two HWDGE queues (sync, scalar)
            # and the SWDGE queue (gpsimd); ordered by when they gate compute.
            sb_xim = consts.tile([48, 256], F32R, tag="xim")
            nc.sync.dma_start(sb_xim[:24, :], x_im.ap()[:24, :])
            nc.scalar.dma_start(sb_xim[24:, :], x_im.ap()[24:, :])
            ident = consts.tile([64, 64], F32R, tag="ident")
            nc.gpsimd.dma_start(ident[:], ident_d.ap())
            sb_w1 = consts.tile([48, 32], F32R, tag="w1")
            nc.gpsimd.dma_start(sb_w1[:], w1r.ap())
            sb_w2 = consts.tile([128, 4, 64], F32R, tag="w2")
            nc.sync.dma_start(sb_w2[:, :2, :], w2k.ap()[:, :2, :])
            nc.scalar.dma_start(sb_w2[:, 2:, :], w2k.ap()[:, 2:, :])
            sb_lkT = consts.tile([64, 512], F32R, tag="lkT")
            nc.gpsimd.dma_start(sb_lkT[:, :256], lkT.ap()[:, :256])
            nc.sync.dma_start(sb_lkT[:, 256:], lkT.ap()[:, 256:])
            sb_wvT = consts.tile([64, 64], F32R, tag="wvT")
            nc.gpsimd.dma_start(sb_wvT[:], wvT.ap())
            sb_wo = consts.tile([64, 64], F32R, tag="wo")
            nc.scalar.dma_start(sb_wo[:], wo.ap())
            if with_bias:
                sb_b1 = consts.tile([32, 1], F32, tag="b1")
                nc.gpsimd.dma_start(sb_b1[:], b1.ap())
                sb_b2 = consts.tile([64, 1], F32, tag="b2")
                nc.gpsimd.dma_start(sb_b2[:], b2.ap())

            # f32r tiles cannot be memset directly; zero/one them via ops
            # from an f32 zero tile (early, no dependencies).
            sb_zero = consts.tile([128, 18, 8], F32, tag="zero")
            nc.vector.memset(sb_zero[:], 0.0)
            sb_one = consts.tile([65, 2], F32R, tag="one")
            nc.vector.tensor_scalar_add(sb_one[64:65, :], sb_zero[64:65, 0, :2], 1.0)

            sb_lk = work.tile([128, 4, 65], F32R, tag="lk")
            nc.vector.tensor_scalar_add(sb_lk[:, :, 64:65],
                                        sb_zero[:, :4, :1], 1.0)

            # ---- conv1: (48,32).T @ (48,256) -> (32, 16, 16) ----
            p_z1 = psA.tile([32, 16, 16], F32, tag="a")
            nc.tensor.matmul(p_z1[:], sb_w1[:], sb_xim[:],
                             start=True, stop=True)

            # ---- conv2 input: imkw[(kw,ci), row, c] = a1pad[ci, row, 2c+kw]
            # where a1pad = zero-pad(relu(z1)).  The ReLU, the shift and the
            # fp32r cast fuse into one tensor_scalar_max per kw, straight
            # from PSUM; pad rows/cols come from the early zero cast-copy.
            def build_imkw(src_psum, imkw):
                nc.vector.tensor_scalar_max(
                    imkw[0:32, 1:17, 1:8], src_psum[:, :, 1:15:2], 0.0)
                nc.vector.tensor_scalar_max(
                    imkw[32:64, 1:17, 0:8], src_psum[:, :, 0:16:2], 0.0)
                nc.vector.tensor_scalar_max(
                    imkw[64:96, 1:17, 0:8], src_psum[:, :, 1:16:2], 0.0)
                nc.vector.tensor_scalar_max(
                    imkw[96:128, 1:17, 0:7], src_psum[:, :, 2:16:2], 0.0)

            def conv2(imkw, ps_tag):
                p = psA.tile([64, 64], F32, tag=ps_tag)
                for kh in range(4):
                    nc.tensor.matmul(
                        p[:],
                        sb_w2[:, kh, :],
                        imkw[:, kh:min(kh + 16, 18):2, :],
                        start=(kh == 0), stop=(kh == 3),
                    )
                return p

            imkw = work.tile([128, 18, 8], F32R, tag="imkw")
            nc.vector.tensor_copy(imkw[:], sb_zero[:])
            if not with_bias:
                build_imkw(p_z1, imkw)
                p_z2 = conv2(imkw, "b")
            else:
                # a1 = relu(z1 + b1); t1m = z1 * sign(a1)
                sb_a1 = work.tile([32, 16, 16], F32, tag="a1")
                nc.scalar.activation(
                    sb_a1[:], p_z1[:], mybir.ActivationFunctionType.Relu,
                    bias=sb_b1[:], scale=1.0,
                )
                sb_m1 = work.tile([32, 16, 16], F32, tag="m1")
                nc.scalar.activation(
                    sb_m1[:], sb_a1[:], mybir.ActivationFunctionType.Sign)
                sb_t1 = work.tile([32, 16, 16], F32, tag="t1")
                nc.vector.tensor_mul(sb_t1[:], p_z1[:], sb_m1[:])

                def shifts(dst, src):
                    nc.vector.tensor_copy(dst[0:32, 1:17, 1:8], src[:, :, 1:15:2])
                    nc.vector.tensor_copy(dst[32:64, 1:17, 0:8], src[:, :, 0:16:2])
                    nc.vector.tensor_copy(dst[64:96, 1:17, 0:8], src[:, :, 1:16:2])
                    nc.vector.tensor_copy(dst[96:128, 1:17, 0:7], src[:, :, 2:16:2])

                shifts(imkw, sb_a1)
                p_z2 = conv2(imkw, "b")
                imkw2 = work.tile([128, 18, 8], F32R, tag="imkw2")
                nc.vector.tensor_copy(imkw2[:], sb_zero[:])
                shifts(imkw2, sb_t1)
                p_t2 = conv2(imkw2, "e")

            sb_zq = work.tile([64, 64], F32R, tag="zq")
            if not with_bias:
                nc.vector.tensor_scalar_max(sb_zq[:], p_z2[:], 0.0)
            else:
                sb_z2r = work.tile([64, 64], F32, tag="z2r")
                nc.scalar.activation(
                    sb_z2r[:], p_z2[:], mybir.ActivationFunctionType.Relu,
                    bias=sb_b2[:], scale=1.0,
                )
                sb_m2 = work.tile([64, 64], F32, tag="m2")
                nc.scalar.activation(
                    sb_m2[:], sb_z2r[:], mybir.ActivationFunctionType.Sign)
                nc.vector.tensor_mul(sb_zq[:], p_t2[:], sb_m2[:])

            # ---- natural-layout lookup chunks from lkT via PE transpose
            # (interleaved with scoresT by DMA-arrival order), plus
            # scoresT: 4 matmuls, (mem128, pos) chunks side by side in one
            # PSUM tile; lkT chunk is the stationary operand.  The lk
            # chunks carry an appended ones-column (written above) so the
            # G matmuls also emit Z.
            p_sT = psA.tile([128, 4, 64], F32, tag="c")
            for c in range(4):
                nc.tensor.matmul(
                    p_sT[:, c, :],
                    sb_lkT[:, 128 * c:128 * (c + 1)], sb_zq[:],
                    start=True, stop=True,
                )
            p_lks = []
            for c in range(4):
                p_lk = psT.tile([128, 64], F32, tag="ptr")
                nc.tensor.matmul(
                    p_lk[:], sb_lkT[:, 128 * c:128 * (c + 1)], ident[:],
                    start=True, stop=True,
                )
                p_lks.append(p_lk)
                nc.scalar.copy(sb_lk[:, c, :64], p_lk[:])

            # ---- Wvo = Wv @ Wo (needed only by the final matmul); the PE
            # is otherwise idle while the exp runs.
            p_wvo = psA.tile([64, 64], F32, tag="d")
            nc.tensor.matmul(p_wvo[:], sb_wvT[:], sb_wo[:],
                             start=True, stop=True)
            sb_wvo = work.tile([64, 64], F32R, tag="wvo")
            nc.scalar.copy(sb_wvo[:], p_wvo[:])

            # unnormalized softmax: E = exp(s/8) over the whole tile.
            # |s/8| << 1 here, so max-subtraction is unnecessary in fp32.
            sb_E = work.tile([128, 4, 64], F32R, tag="E")
            nc.scalar.activation(
                sb_E[:], p_sT[:], mybir.ActivationFunctionType.Exp,
                scale=0.125,
            )

            # ---- [G; Z][d, pos] = sum_m [lk | 1][m, d] * E[m, pos] ----
            p_g = psA.tile([65, 64], F32, tag="d")
            for c in range(4):
                nc.tensor.matmul(
                    p_g[:], sb_lk[:, c, :], sb_E[:, c, :],
                    start=(c == 0), stop=(c == 3),
                )
            sb_g = work.tile([65, 64], F32R, tag="g")
            nc.vector.tensor_copy(sb_g[:], p_g[:])

            # Z row -> per-partition column via a K=1 matmul, then 1/Z
            p_zT = psA.tile([64, 2], F32, tag="b")
            nc.tensor.matmul(p_zT[:], sb_g[64:65, :].bitcast(F32), sb_one[64:65, :].bitcast(F32),
                             start=True, stop=True)
            sb_rz = work.tile([64, 1], F32, tag="rz")
            nc.vector.reciprocal(sb_rz[:], p_zT[:, :1])

            # ---- out2[pos, ch'] = (G.T @ Wvo)[pos, ch'] / Z[pos] ----
            p_o = psA.tile([64, 64], F32, tag="a")
            nc.tensor.matmul(p_o[:], sb_g[:64, :], sb_wvo[:],
                             start=True, stop=True)
            sb_out = work.tile([64, 64], F32, tag="out")
            nc.vector.tensor_scalar_mul(sb_out[:], p_o[:], sb_rz[:])
            nc.sync.dma_start(out_d.ap()[:32, :], sb_out[:32, :])
            nc.scalar.dma_start(out_d.ap()[32:, :], sb_out[32:, :])

    nc.compile()
    return nc


def _get_nc(with_bias: bool):
    if with_bias not in _COMPILED:
        _COMPILED[with_bias] = _build(with_bias)
    return _COMPILED[with_bias]


def kernel(x, conv1_w, conv1_b, conv2_w, conv2_b, lookup, Wv, Wo):
    global last_exec_time_ns, last_trace_path
    x = np.asarray(x, np.float32)
    w1 = np.asarray(conv1_w, np.float32)
    b1 = np.asarray(conv1_b, np.float32)
    w2 = np.asarray(conv2_w, np.float32)
    b2 = np.asarray(conv2_b, np.float32)
    lk = np.ascontiguousarray(np.asarray(lookup, np.float32))
    wv = np.ascontiguousarray(np.asarray(Wv, np.float32))
    wo = np.ascontiguousarray(np.asarray(Wo, np.float32))

    with_bias = bool(np.any(b1 != 0.0) or np.any(b2 != 0.0))

    # host-side layout prep (no arithmetic): im2col of padded x, weight
    # transposes to the matmul-native layouts.
    xp = np.zeros((B, CIN, 34, 34), np.float32)
    xp[:, :, 1:33, 1:33] = x
    xim = np.empty((B, CIN, 4, 4, 16, 16), np.float32)
    for kh in range(4):
        for kw in range(4):
            xim[:, :, kh, kw] = xp[:, :, kh:kh + 32:2, kw:kw + 32:2]
    xim = np.ascontiguousarray(xim.reshape(B, 48, 256))

    w1r = np.ascontiguousarray(w1.transpose(1, 2, 3, 0).reshape(48, 32))
    # w2k[(kw*32+ci), kh, co] = w2[co, ci, kh, kw]
    w2k = np.ascontiguousarray(w2.transpose(3, 1, 2, 0).reshape(128, 4, 64))
    lkT = np.ascontiguousarray(lk.T)
    wvT = np.ascontiguousarray(wv.T)

    shared = {"w1r": w1r, "w2k": w2k, "lkT": lkT, "wvT": wvT, "wo": wo,
              "ident": np.eye(64, dtype=np.float32)}
    if with_bias:
        shared["b1"] = np.ascontiguousarray(b1.reshape(32, 1))
        shared["b2"] = np.ascontiguousarray(b2.reshape(64, 1))

    in_maps = [dict(shared, x_im=xim[c % B]) for c in range(N_CORES)]

    nc = _get_nc(with_bias)
    trace = bool(os.environ.get("KERNEL_TRACE"))
    res = run_bass_kernel_spmd(
        nc, in_maps, core_ids=list(range(N_CORES)),
        trace=trace, trace_cores=[0] if trace else None,
    )
    last_exec_time_ns = res.exec_time_ns
    if res.instructions_and_trace:
        last_trace_path = res.instructions_and_trace[1]

    # device emits (pos, ch') per sample; host transposes (layout only)
    out = np.stack([res.results[b]["out"].T for b in range(B)])
    return np.ascontiguousarray(out.reshape(B, C2, 8, 8))
